# revision 7
# baseline (speedup 1.0000x reference)
"""BatchedPeriodicDistance on 8 Trainium2 NeuronCores (Bass/Tile).

Strategy (per the edge-sharding hint):
  - Edges are sharded 8 ways; pos/box/batch replicated to every core.
  - Per-box phase (on device): 3x3 inverse via adjugate -> BC table
    [128 boxes, 20 f32] = [box rows | inv rows | pad] in DRAM scratch.
  - Per-atom phase (on device): `batch` is sorted, so each partition's
    contiguous 784-atom range only spans a few box ids.  Box/inv rows are
    fetched per-partition with K candidate indirect-DMA gathers
    (K = max span, computed host-side from the index structure) and
    per-atom coefficients built with masked accumulation.  Computes
    wrapped positions posw = ((pos @ inv) mod 1) @ box and writes the
    per-atom table W[atom] = [posw(3), pad, box rows(9), pad(3)] (64B).
  - Per-edge phase (on device): two dma_gather streams per edge chunk
    fetch 256B quads (4 atom rows) for i0 (full row needed) and i1
    (posw only) with int16 indices = atom_id >> 2; a 4-way predicated
    select picks the right 64B sub-row; DVE computes
    ev = posw[i0] - posw[i1] - sum_d s_d * boxrow_d  and  ew = |ev|.

Self-contained: only concourse / numpy imports, all shapes derived from
the inputs.
"""

import os

import numpy as np

import concourse.bacc as bacc
import concourse.bass as bass
import concourse.tile as tile
import concourse.mybir as mybir
from concourse import library_config
from concourse.bass_utils import run_bass_kernel_spmd

F32 = mybir.dt.float32
I32 = mybir.dt.int32
I16 = mybir.dt.int16
ALU = mybir.AluOpType

N_CORES = 8
P = 128                 # partitions
EN = 8192               # edges per gather chunk
EC = EN // P            # 64 edge slots per partition per chunk
EJ = EN // 16           # 512 wrapped-idx columns
MAGIC = 8388608.0       # 2^23, round-to-nearest trick for floor/mod

LAST_EXEC_NS = None
LAST_MEAN_EXEC_NS = None


def _build_kernel(n_chunks, atoms_pp, n_boxes, kcand, debug_w=False):
    """Build the SPMD Bass program (identical on all cores)."""
    npad = P * atoms_pp
    nquad = npad // 4
    af = atoms_pp  # atoms per partition (free dim of atom phase)

    nc = bacc.Bacc("TRN2", target_bir_lowering=False, num_swdge_queues=4)

    # ---- I/O ----
    pos_t = nc.dram_tensor("pos_t", [3, P, af], F32, kind="ExternalInput")
    batch_n = nc.dram_tensor("batch_n", [P, af], I32, kind="ExternalInput")
    bstart = nc.dram_tensor("bstart", [P, 1], I32, kind="ExternalInput")
    box_r = nc.dram_tensor("box_r", [P, 9], F32, kind="ExternalInput")
    i0w = nc.dram_tensor("i0w", [n_chunks, P, EJ], I32, kind="ExternalInput")
    i1w = nc.dram_tensor("i1w", [n_chunks, P, EJ], I32, kind="ExternalInput")
    i0n = nc.dram_tensor("i0n", [n_chunks, P, EC], I32, kind="ExternalInput")
    i1n = nc.dram_tensor("i1n", [n_chunks, P, EC], I32, kind="ExternalInput")
    sn = nc.dram_tensor("sn", [n_chunks, P, EC, 3], F32, kind="ExternalInput")
    ew = nc.dram_tensor("ew", [n_chunks, P, EC], F32, kind="ExternalOutput")
    ev = nc.dram_tensor("ev", [n_chunks, P, EC, 3], F32, kind="ExternalOutput")

    # ---- DRAM scratch ----
    # NB: per-core DRAM scratch must be ExternalOutput: Internal scratchpad
    # allocations collide across the 8 SPMD cores (runtime-computed gather
    # addresses read another core's half-written pages).
    bc_d = nc.dram_tensor("bc_d", [n_boxes, 20], F32, kind="ExternalOutput")
    wq = nc.dram_tensor("wq", [nquad, 64], F32, kind="ExternalOutput")
    # per-atom row view of the quad table: [P, af, 16]
    w_rows = wq[:].rearrange("q (r w) -> (q r) w", r=4).rearrange(
        "(p f) w -> p f w", p=P)

    with tile.TileContext(nc) as tc:
        with (
            tc.tile_pool(name="c0", bufs=1) as c0,          # one-shot tiles
            tc.tile_pool(name="atom", bufs=1) as ap_,       # atom phase
        ):
            nc.gpsimd.load_library(library_config.mlp)

            # ================= Phase 0: box inverse + BC table ============
            bx = c0.tile([P, 9], F32)
            nc.sync.dma_start(bx[:], box_r[:])
            num = c0.tile([P, 9], F32)   # inv numerators (adjugate)
            t0 = c0.tile([P, 1], F32)
            t1 = c0.tile([P, 1], F32)
            m = lambda r, c: bx[:, 3 * r + c : 3 * r + c + 1]

            def minor(out, a, b, c, d):
                # out = m[a]*m[b] - m[c]*m[d]  (flat col indices)
                nc.vector.tensor_tensor(t0[:], bx[:, a:a+1], bx[:, b:b+1], ALU.mult)
                nc.vector.tensor_tensor(t1[:], bx[:, c:c+1], bx[:, d:d+1], ALU.mult)
                nc.vector.tensor_tensor(out, t0[:], t1[:], ALU.subtract)

            # row-major m[r][c] at col 3r+c ; inv numerators N[i][j]:
            minor(num[:, 0:1], 4, 8, 5, 7)   # N00 = m11m22-m12m21
            minor(num[:, 1:2], 2, 7, 1, 8)   # N01 = m02m21-m01m22
            minor(num[:, 2:3], 1, 5, 2, 4)   # N02 = m01m12-m02m11
            minor(num[:, 3:4], 5, 6, 3, 8)   # N10 = m12m20-m10m22
            minor(num[:, 4:5], 0, 8, 2, 6)   # N11 = m00m22-m02m20
            minor(num[:, 5:6], 2, 3, 0, 5)   # N12 = m02m10-m00m12
            minor(num[:, 6:7], 3, 7, 4, 6)   # N20 = m10m21-m11m20
            minor(num[:, 7:8], 1, 6, 0, 7)   # N21 = m01m20-m00m21
            minor(num[:, 8:9], 0, 4, 1, 3)   # N22 = m00m11-m01m10
            det = c0.tile([P, 1], F32)
            nc.vector.tensor_tensor(det[:], m(0, 0), num[:, 0:1], ALU.mult)
            nc.vector.tensor_tensor(t0[:], m(0, 1), num[:, 3:4], ALU.mult)
            nc.vector.tensor_tensor(det[:], det[:], t0[:], ALU.add)
            nc.vector.tensor_tensor(t0[:], m(0, 2), num[:, 6:7], ALU.mult)
            nc.vector.tensor_tensor(det[:], det[:], t0[:], ALU.add)
            rdet = c0.tile([P, 1], F32)
            nc.vector.reciprocal(rdet[:], det[:])

            bc = c0.tile([P, 20], F32)
            nc.vector.memset(bc[:], 0.0)
            nc.vector.tensor_copy(bc[:, 0:9], bx[:])
            nc.vector.tensor_scalar(bc[:, 9:18], num[:], rdet[:, 0:1], None, ALU.mult)
            nc.sync.dma_start(bc_d[:], bc[:])

            # ================= Phase 1: per-atom wrap + W table ===========
            xt = ap_.tile([P, af], F32)
            yt = ap_.tile([P, af], F32)
            zt = ap_.tile([P, af], F32)
            nc.sync.dma_start(xt[:], pos_t[0])
            nc.sync.dma_start(yt[:], pos_t[1])
            nc.sync.dma_start(zt[:], pos_t[2])
            bt = ap_.tile([P, af], I32)
            nc.sync.dma_start(bt[:], batch_n[:])
            btf = ap_.tile([P, af], F32)
            nc.vector.tensor_copy(btf[:], bt[:])
            bst = ap_.tile([P, 1], I32)
            nc.sync.dma_start(bst[:], bstart[:])
            bstf = ap_.tile([P, 1], F32)
            nc.vector.tensor_copy(bstf[:], bst[:])

            # K candidate gathers of BC rows (bstart+k), per-partition scalars
            bck = []
            for k in range(kcand):
                bpk = ap_.tile([P, 1], I32, tag=f"bpk{k}")
                nc.vector.tensor_scalar(bpk[:], bst[:], k, n_boxes - 1,
                                        ALU.add, ALU.min)
                g = ap_.tile([P, 20], F32, tag=f"bck{k}")
                nc.gpsimd.indirect_dma_start(
                    out=g[:], out_offset=None, in_=bc_d[:],
                    in_offset=bass.IndirectOffsetOnAxis(ap=bpk[:, 0:1], axis=0))
                bck.append(g)

            wp = ap_.tile([P, af, 16], F32)
            nc.vector.memset(wp[:], 0.0)

            # coefficient tiles: inv (9) accumulated into cinv, box rows (9)
            # accumulated directly into wp cols 4..12
            cinv = [ap_.tile([P, af], F32, name=f"cinv{ij}", tag=f"cinv{ij}")
                    for ij in range(9)]
            for k in range(kcand):
                mk = ap_.tile([P, af], F32, tag="mk")
                bkf = ap_.tile([P, 1], F32, tag="bkf")
                nc.vector.tensor_scalar(bkf[:], bstf[:], float(k), None, ALU.add)
                nc.vector.tensor_scalar(mk[:], btf[:], bkf[:, 0:1], None, ALU.is_equal)
                for ij in range(9):
                    sc = bck[k][:, 9 + ij : 10 + ij]
                    if k == 0:
                        nc.vector.tensor_scalar(cinv[ij][:], mk[:], sc, None, ALU.mult)
                    else:
                        nc.vector.scalar_tensor_tensor(
                            cinv[ij][:], mk[:], sc, cinv[ij][:], ALU.mult, ALU.add)
                for ij in range(9):
                    sc = bck[k][:, ij : ij + 1]
                    dst = wp[:, :, 4 + ij]
                    if k == 0:
                        nc.vector.tensor_scalar(dst, mk[:], sc, None, ALU.mult)
                    else:
                        nc.vector.scalar_tensor_tensor(
                            dst, mk[:], sc, dst, ALU.mult, ALU.add)

            # frac_j = x*cinv[0j] + y*cinv[1j] + z*cinv[2j] ; wrap ; posw
            ta = ap_.tile([P, af], F32)
            tb = ap_.tile([P, af], F32)
            fw = [ap_.tile([P, af], F32, name=f"fw{j}", tag=f"fw{j}")
                  for j in range(3)]
            for j in range(3):
                fr = ap_.tile([P, af], F32, tag="fr")
                nc.vector.tensor_tensor(fr[:], xt[:], cinv[j][:], ALU.mult)
                nc.vector.tensor_tensor(ta[:], yt[:], cinv[3 + j][:], ALU.mult)
                nc.vector.tensor_tensor(fr[:], fr[:], ta[:], ALU.add)
                nc.vector.tensor_tensor(ta[:], zt[:], cinv[6 + j][:], ALU.mult)
                nc.vector.tensor_tensor(fr[:], fr[:], ta[:], ALU.add)
                # this jax build's `frac % 1.0` is IEEE-remainder-style:
                # fw = fr - round_half_even(fr)  (can be negative)
                nc.vector.tensor_scalar(ta[:], fr[:], MAGIC, -MAGIC, ALU.add, ALU.add)
                nc.vector.tensor_tensor(fw[j][:], fr[:], ta[:], ALU.subtract)
            for j in range(3):
                dst = wp[:, :, j]
                nc.vector.tensor_tensor(ta[:], fw[0][:], wp[:, :, 4 + j], ALU.mult)
                nc.vector.tensor_tensor(tb[:], fw[1][:], wp[:, :, 7 + j], ALU.mult)
                nc.vector.tensor_tensor(ta[:], ta[:], tb[:], ALU.add)
                nc.vector.tensor_tensor(tb[:], fw[2][:], wp[:, :, 10 + j], ALU.mult)
                nc.vector.tensor_tensor(dst, ta[:], tb[:], ALU.add)

            nc.sync.dma_start(w_rows, wp[:])

        # ================= Phase 2: edges =================================
        with (
            tc.tile_pool(name="eio", bufs=3) as eio,        # edge chunk inputs
            tc.tile_pool(name="eg", bufs=2) as eg,          # gather dests
            tc.tile_pool(name="ec", bufs=2) as ecp,         # edge compute
        ):
            for t in range(n_chunks):
                iw0 = eio.tile([P, EJ], I32, tag="iw0")
                nc.sync.dma_start(iw0[:], i0w[t])
                iw1 = eio.tile([P, EJ], I32, tag="iw1")
                nc.sync.dma_start(iw1[:], i1w[t])
                in0 = eio.tile([P, EC], I32, tag="in0")
                nc.sync.dma_start(in0[:], i0n[t])
                in1 = eio.tile([P, EC], I32, tag="in1")
                nc.sync.dma_start(in1[:], i1n[t])
                st = eio.tile([P, EC, 3], F32, tag="st")
                nc.sync.dma_start(st[:], sn[t])

                ish = ecp.tile([P, EJ], I32, tag="ish")
                q0 = ecp.tile([P, EJ], I16, tag="q0")
                nc.vector.tensor_scalar(ish[:], iw0[:], 2, None, ALU.arith_shift_right)
                nc.vector.tensor_copy(q0[:], ish[:])
                ish1 = ecp.tile([P, EJ], I32, tag="ish1")
                q1 = ecp.tile([P, EJ], I16, tag="q1")
                nc.vector.tensor_scalar(ish1[:], iw1[:], 2, None, ALU.arith_shift_right)
                nc.vector.tensor_copy(q1[:], ish1[:])

                g0 = eg.tile([P, EC, 64], F32, tag="g0")
                nc.gpsimd.dma_gather(g0[:], wq[:], q0[:], EN, EN, 64,
                                     single_packet=False, queue_num=(2 * t) % 4)
                g1 = eg.tile([P, EC, 64], F32, tag="g1")
                nc.gpsimd.dma_gather(g1[:], wq[:], q1[:], EN, EN, 64,
                                     single_packet=False, queue_num=(2 * t + 1) % 4)

                # 4-way sub-row select (quad position = atom & 3)
                def subsel(dst, graw, inn, width, tagp):
                    a3 = ecp.tile([P, EC], I32, tag=f"{tagp}a3", name=f"{tagp}a3")
                    nc.vector.tensor_scalar(a3[:], inn[:], 3, None, ALU.bitwise_and)
                    nc.vector.tensor_copy(dst, graw[:, :, 0:width])
                    for k in range(1, 4):
                        mk = ecp.tile([P, EC], I32, tag=f"{tagp}mk", name=f"{tagp}mk")
                        nc.vector.tensor_scalar(mk[:], a3[:], k, None,
                                                ALU.is_equal)
                        mkb = mk[:].unsqueeze(2).broadcast_to([P, EC, width])
                        nc.vector.copy_predicated(
                            dst, mkb, graw[:, :, 16 * k : 16 * k + width])

                g0s = ecp.tile([P, EC, 16], F32, tag="g0s")
                subsel(g0s[:], g0, in0, 16, "s0")
                g1s = ecp.tile([P, EC, 4], F32, tag="g1s")
                subsel(g1s[:], g1, in1, 4, "s1")

                # ev = (posw0 - posw1) - sum_d s_d * boxrow_d
                evp = ecp.tile([P, EC, 3], F32, tag="evp")
                ta2 = ecp.tile([P, EC, 3], F32, tag="ta2")
                tb2 = ecp.tile([P, EC, 3], F32, tag="tb2")
                nc.vector.tensor_tensor(evp[:], g0s[:, :, 0:3], g1s[:, :, 0:3],
                                        ALU.subtract)
                for d in range(3):
                    sd = st[:, :, d].unsqueeze(2).broadcast_to([P, EC, 3])
                    dst = ta2 if d == 0 else tb2
                    nc.vector.tensor_tensor(dst[:], sd,
                                            g0s[:, :, 4 + 3 * d : 7 + 3 * d],
                                            ALU.mult)
                    if d > 0:
                        nc.vector.tensor_tensor(ta2[:], ta2[:], tb2[:], ALU.add)
                nc.vector.tensor_tensor(evp[:], evp[:], ta2[:], ALU.subtract)

                sq = ecp.tile([P, EC, 3], F32, tag="sq")
                nc.vector.tensor_tensor(sq[:], evp[:], evp[:], ALU.mult)
                ss = ecp.tile([P, EC], F32, tag="ss")
                nc.vector.tensor_reduce(ss[:], sq[:], mybir.AxisListType.X, ALU.add)
                wt_ = ecp.tile([P, EC], F32, tag="wt")
                nc.scalar.sqrt(wt_[:], ss[:])

                nc.sync.dma_start(ew[t], wt_[:])
                nc.sync.dma_start(ev[t], evp[:])

    nc.compile()
    return nc


def kernel(pos, box, batch, precomputed_edge_index, precomputed_shifts_idx):
    pos = np.ascontiguousarray(np.asarray(pos, dtype=np.float32))
    box = np.ascontiguousarray(np.asarray(box, dtype=np.float32))
    batch = np.ascontiguousarray(np.asarray(batch, dtype=np.int32))
    eidx = np.asarray(precomputed_edge_index, dtype=np.int32)
    sidx = np.asarray(precomputed_shifts_idx, dtype=np.float32)

    n_atoms = pos.shape[0]
    n_boxes = box.shape[0]
    n_edges = eidx.shape[1]

    # ---- atom padding: P partitions x atoms_pp contiguous atoms ----
    atoms_pp = -(-n_atoms // P)          # 784 for 100000
    atoms_pp = -(-atoms_pp // 4) * 4     # keep quad rows aligned
    npad = P * atoms_pp
    pos_pad = np.zeros((npad, 3), np.float32)
    pos_pad[:n_atoms] = pos
    batch_pad = np.full(npad, int(batch[-1]), np.int32)
    batch_pad[:n_atoms] = batch
    pos_t = np.ascontiguousarray(
        pos_pad.reshape(P, atoms_pp, 3).transpose(2, 0, 1))
    batch_n = batch_pad.reshape(P, atoms_pp)
    bstart = np.ascontiguousarray(batch_n[:, :1])
    # K candidates: max number of distinct box ids in any partition range
    spans = (batch_n.max(axis=1) - batch_n[:, 0]).max() + 1
    kcand = int(spans)
    box_r = np.ascontiguousarray(box.reshape(n_boxes, 9))

    # ---- edge sharding / padding ----
    e_core = -(-n_edges // N_CORES)
    n_chunks = -(-e_core // EN)
    e_core_pad = n_chunks * EN

    def shard_edges(core):
        lo = core * e_core
        hi = min(lo + e_core, n_edges)
        ln = hi - lo
        i0 = np.zeros(e_core_pad, np.int32)
        i1 = np.zeros(e_core_pad, np.int32)
        ss = np.zeros((e_core_pad, 3), np.float32)
        i0[:ln] = eidx[0, lo:hi]
        i1[:ln] = eidx[1, lo:hi]
        ss[:ln] = sidx[lo:hi]
        # natural layout: chunk t, partition p, slot s  <- edge t*EN + p*EC + s
        i0n = i0.reshape(n_chunks, P, EC)
        i1n = i1.reshape(n_chunks, P, EC)
        sn = ss.reshape(n_chunks, P, EC, 3)

        def wrap(xn):  # [T, 128, EC] -> [T, 128, EJ] wrapped + replicated
            w = xn.reshape(n_chunks, 8, 16, EC).transpose(0, 2, 3, 1)
            w = np.ascontiguousarray(w.reshape(n_chunks, 16, 8 * EC))
            return np.ascontiguousarray(
                np.broadcast_to(w[:, None], (n_chunks, 8, 16, 8 * EC))
                .reshape(n_chunks, P, EJ))

        return {
            "pos_t": pos_t, "batch_n": batch_n, "bstart": bstart,
            "box_r": box_r,
            "i0w": wrap(i0n), "i1w": wrap(i1n),
            "i0n": np.ascontiguousarray(i0n),
            "i1n": np.ascontiguousarray(i1n),
            "sn": np.ascontiguousarray(sn),
        }

    in_maps = [shard_edges(c) for c in range(N_CORES)]

    nc = _build_kernel(n_chunks, atoms_pp, n_boxes, kcand)
    res = run_bass_kernel_spmd(nc, in_maps, core_ids=list(range(N_CORES)))

    global LAST_EXEC_NS, LAST_MEAN_EXEC_NS
    LAST_EXEC_NS = res.exec_time_ns
    LAST_MEAN_EXEC_NS = res.mean_exec_time_ns

    edge_weight = np.empty(n_edges, np.float32)
    edge_vec = np.empty((n_edges, 3), np.float32)
    for c in range(N_CORES):
        lo = c * e_core
        hi = min(lo + e_core, n_edges)
        ln = hi - lo
        ewc = res.results[c]["ew"].reshape(e_core_pad)
        evc = res.results[c]["ev"].reshape(e_core_pad, 3)
        edge_weight[lo:hi] = ewc[:ln]
        edge_vec[lo:hi] = evc[:ln]

    return (eidx, edge_weight, edge_vec, sidx)
